# revision 13
# baseline (speedup 1.0000x reference)
"""Trainium2 Bass kernel for nn_Loca_901943132312 (loss_fn).

Per row i of teacher_logits [4096, 32000]:
    S = sum_j logits[i, j]
    t = logits[i, label_i]
    s = 0.95 / (1 + S - 2 t)
    out[i, j]       = s * logits[i, j]      (j != label)
    out[i, label_i] = 1 - s * S + s * t

Data-parallel across 8 NeuronCores: 512 rows per core. Per core the rows
map to partitions (4 blocks of 128); the 32000-wide free dim streams
through SBUF in chunks of 4000 f32 that stay resident for one block so
HBM is read once and written once (memory-bound roofline ~131 MB/core,
~308 us at the ~425 GB/s per-core DMA ceiling). DVE does the row-sum
reduction and the tiny stats chain, ACT does the per-row rescale, the
label element is gathered/fixed up with indirect DMA ordered after the
bulk stores.
"""

import sys

import numpy as np

try:
    import concourse.bacc as bacc
except ModuleNotFoundError:
    sys.path.insert(0, "/opt/trn_rl_repo")
    import concourse.bacc as bacc
import concourse.tile as tile
from concourse import bass, mybir
import concourse.bass_utils as bass_utils
from concourse.bass_utils import run_bass_kernel_spmd
from concourse.tile_rust import add_dep_helper

# If tracing is ever enabled (e.g. BASS_TRACE in the environment), don't let
# an unreachable artifact store kill the run.
_orig_upload = bass_utils.upload_artifacts


def _safe_upload(tmpdir):
    try:
        return _orig_upload(tmpdir)
    except Exception:
        return "local://" + tmpdir


bass_utils.upload_artifacts = _safe_upload

ALPHA = 0.95
B, C = 4096, 32000
N_CORES = 8
BS = B // N_CORES  # rows per core
P = 128
NBLK = BS // P  # row blocks per core
F = 4000  # chunk width (free dim)
NCH = C // F  # chunks per block
DATA_BUFS = NCH + 3  # resident block + lookahead into next block

_CACHE = {}


def _raw(inst):
    return inst.ins if isinstance(inst, bass.BassInstruction) else inst


def _build():
    nc = bacc.Bacc(
        "TRN2", target_bir_lowering=False, debug=False, num_devices=N_CORES
    )
    lg = nc.dram_tensor("logits", [BS * C], mybir.dt.float32, kind="ExternalInput").ap()
    offs = nc.dram_tensor("offs", [P, NBLK], mybir.dt.int32, kind="ExternalInput").ap()
    out = nc.dram_tensor("out", [BS * C], mybir.dt.float32, kind="ExternalOutput").ap()

    lg2 = lg.rearrange("(r c) -> r c", c=C)
    out2 = out.rearrange("(r c) -> r c", c=C)
    lgN1 = lg.rearrange("(n one) -> n one", one=1)
    outN1 = out.rearrange("(n one) -> n one", one=1)

    fp32 = mybir.dt.float32
    X = mybir.AxisListType.X

    with tile.TileContext(nc) as tc:
        with (
            tc.tile_pool(name="data", bufs=DATA_BUFS) as data,
            tc.tile_pool(name="stats", bufs=2) as stats,
            tc.tile_pool(name="singles", bufs=1) as singles,
        ):
            offs_t = singles.tile([P, NBLK], mybir.dt.int32)
            t_all = singles.tile([P, NBLK], fp32)

            for b in range(NBLK):
                rows = slice(b * P, (b + 1) * P)
                # One extra column: the last chunk loads/reduces as two
                # halves so its reduce overlaps the second half's DMA and
                # S is ready sooner at the block transition.
                sparts = stats.tile([P, NCH + 1], fp32)
                chunks = []
                for k in range(NCH):
                    ck = data.tile([P, F], fp32, tag="data")
                    if k == NCH - 1:
                        H = F // 2
                        for h in range(2):
                            nc.sync.dma_start(
                                out=ck[:, h * H : (h + 1) * H],
                                in_=lg2[rows, k * F + h * H : k * F + (h + 1) * H],
                            )
                            nc.vector.reduce_sum(
                                out=sparts[:, k + h : k + h + 1],
                                in_=ck[:, h * H : (h + 1) * H],
                                axis=X,
                            )
                    else:
                        nc.sync.dma_start(
                            out=ck[:], in_=lg2[rows, k * F : (k + 1) * F]
                        )
                        nc.vector.reduce_sum(
                            out=sparts[:, k : k + 1], in_=ck[:], axis=X
                        )
                    chunks.append(ck)

                if b == 0:
                    # Emitted after block 0's loads so the first bulk load is
                    # the first thing on the sync ring; gathers only need the
                    # offsets and run on gpsimd while loads stream.
                    nc.sync.dma_start(out=offs_t[:], in_=offs[:])
                    for bb in range(NBLK):
                        nc.gpsimd.indirect_dma_start(
                            out=t_all[:, bb : bb + 1],
                            out_offset=None,
                            in_=lgN1[:],
                            in_offset=bass.IndirectOffsetOnAxis(
                                ap=offs_t[:, bb : bb + 1], axis=0
                            ),
                        )

                S = stats.tile([P, 1], fp32)
                nc.vector.reduce_sum(out=S[:], in_=sparts[:], axis=X)

                # s = ALPHA / (1 + S - 2 t)  ==  1 / ((1+S)/ALPHA - (2/ALPHA) t)
                e1 = stats.tile([P, 1], fp32)
                nc.vector.tensor_scalar(
                    out=e1[:], in0=S[:], scalar1=1.0 / ALPHA, scalar2=1.0 / ALPHA,
                    op0=mybir.AluOpType.mult, op1=mybir.AluOpType.add,
                )
                d1 = stats.tile([P, 1], fp32)
                nc.vector.tensor_scalar(
                    out=d1[:], in0=t_all[:, b : b + 1], scalar1=-2.0 / ALPHA,
                    scalar2=e1[:],
                    op0=mybir.AluOpType.mult, op1=mybir.AluOpType.add,
                )
                s_t = stats.tile([P, 1], fp32)
                nc.vector.reciprocal(out=s_t[:], in_=d1[:])

                # val = s*t + (1 - s*S)   (the corrected out[i, label])
                sS = stats.tile([P, 1], fp32)
                nc.vector.tensor_mul(out=sS[:], in0=s_t[:], in1=S[:])
                corr = stats.tile([P, 1], fp32)
                nc.vector.tensor_scalar(
                    out=corr[:], in0=sS[:], scalar1=-1.0, scalar2=1.0,
                    op0=mybir.AluOpType.mult, op1=mybir.AluOpType.add,
                )
                val = stats.tile([P, 1], fp32)
                nc.vector.tensor_scalar(
                    out=val[:], in0=t_all[:, b : b + 1], scalar1=s_t[:],
                    scalar2=corr[:],
                    op0=mybir.AluOpType.mult, op1=mybir.AluOpType.add,
                )

                store_insts = []
                for k, ck in enumerate(chunks):
                    if k == 0:
                        # Split the first rescale so the first store issues
                        # one half-chunk of ACT latency sooner.
                        H = F // 2
                        for h in range(2):
                            cols = slice(h * H, (h + 1) * H)
                            nc.scalar.mul(
                                out=ck[:, cols], in_=ck[:, cols], mul=s_t[:]
                            )
                            si = nc.scalar.dma_start(
                                out=out2[rows, k * F + h * H : k * F + (h + 1) * H],
                                in_=ck[:, cols],
                            )
                            store_insts.append(si)
                    else:
                        nc.scalar.mul(out=ck[:], in_=ck[:], mul=s_t[:])
                        si = nc.scalar.dma_start(
                            out=out2[rows, k * F : (k + 1) * F], in_=ck[:]
                        )
                        store_insts.append(si)

                sc = nc.gpsimd.indirect_dma_start(
                    out=outN1[:],
                    out_offset=bass.IndirectOffsetOnAxis(
                        ap=offs_t[:, b : b + 1], axis=0
                    ),
                    in_=val[:],
                    in_offset=None,
                )
                # The scatter overwrites out[i, label] after the bulk store of
                # the same rows; Tile doesn't order DRAM WAW, so force it.
                for si in store_insts:
                    add_dep_helper(
                        _raw(sc), _raw(si), reason="label fixup after bulk store"
                    )

    nc.compile()
    return nc


def _get_nc():
    if "nc" not in _CACHE:
        _CACHE["nc"] = _build()
    return _CACHE["nc"]


def _shard(teacher_logits, true_labels):
    lg = np.asarray(teacher_logits, dtype=np.float32)
    lab = np.asarray(true_labels).astype(np.int64)
    assert lg.shape == (B, C) and lab.shape == (B,)
    local_rows = np.arange(BS, dtype=np.int64)
    in_maps = []
    for c in range(N_CORES):
        shard = np.ascontiguousarray(lg[c * BS : (c + 1) * BS]).reshape(-1)
        flat = local_rows * C + lab[c * BS : (c + 1) * BS]
        offs_mat = np.ascontiguousarray(
            flat.astype(np.int32).reshape(NBLK, P).T
        )
        in_maps.append({"logits": shard, "offs": offs_mat})
    return in_maps


def _run(teacher_logits, true_labels, **kwargs):
    nc = _get_nc()
    in_maps = _shard(teacher_logits, true_labels)
    res = run_bass_kernel_spmd(nc, in_maps, core_ids=list(range(N_CORES)), **kwargs)
    out = np.concatenate(
        [res.results[c]["out"].reshape(BS, C) for c in range(N_CORES)], axis=0
    )
    return out, res


def kernel(teacher_logits, true_labels):
    return _run(teacher_logits, true_labels)[0]


if __name__ == "__main__":
    rng = np.random.default_rng(0)
    lg = rng.random((B, C), dtype=np.float32)
    lab = rng.integers(0, C, size=(B,), dtype=np.int64)
    got = kernel(lg, lab)
    S = lg.sum(axis=1)
    t = lg[np.arange(B), lab]
    s = ALPHA / (1.0 + S - 2.0 * t)
    want = s[:, None] * lg
    want[np.arange(B), lab] += 1.0 - s * S
    err = np.abs(got - want).max() / np.abs(want).max()
    print("self-check rel err:", err)


# revision 14
# speedup vs baseline: 1.0376x; 1.0376x over previous
"""Trainium2 Bass kernel for nn_Loca_901943132312 (loss_fn).

Per row i of teacher_logits [4096, 32000]:
    S = sum_j logits[i, j]
    t = logits[i, label_i]
    s = 0.95 / (1 + S - 2 t)
    out[i, j]       = s * logits[i, j]      (j != label)
    out[i, label_i] = 1 - s * S + s * t

Data-parallel across 8 NeuronCores: 512 rows per core. Per core the rows
map to partitions (4 blocks of 128); the 32000-wide free dim streams
through SBUF in chunks of 4000 f32 that stay resident for one block so
HBM is read once and written once (memory-bound roofline ~131 MB/core,
~308 us at the ~425 GB/s per-core DMA ceiling). DVE does the row-sum
reduction and the tiny stats chain, ACT does the per-row rescale, the
label element is gathered/fixed up with indirect DMA ordered after the
bulk stores.
"""

import sys

import numpy as np

try:
    import concourse.bacc as bacc
except ModuleNotFoundError:
    sys.path.insert(0, "/opt/trn_rl_repo")
    import concourse.bacc as bacc
import concourse.tile as tile
from concourse import bass, mybir
import concourse.bass_utils as bass_utils
from concourse.bass_utils import run_bass_kernel_spmd
from concourse.tile_rust import add_dep_helper

# If tracing is ever enabled (e.g. BASS_TRACE in the environment), don't let
# an unreachable artifact store kill the run.
_orig_upload = bass_utils.upload_artifacts


def _safe_upload(tmpdir):
    try:
        return _orig_upload(tmpdir)
    except Exception:
        return "local://" + tmpdir


bass_utils.upload_artifacts = _safe_upload

ALPHA = 0.95
B, C = 4096, 32000
N_CORES = 8
BS = B // N_CORES  # rows per core
P = 128
NBLK = BS // P  # row blocks per core
F = 4000  # chunk width (free dim)
NCH = C // F  # chunks per block
DATA_BUFS = NCH + 3  # resident block + lookahead into next block

_CACHE = {}


def _raw(inst):
    return inst.ins if isinstance(inst, bass.BassInstruction) else inst


def _build():
    nc = bacc.Bacc(
        "TRN2", target_bir_lowering=False, debug=False, num_devices=N_CORES
    )
    lg = nc.dram_tensor("logits", [BS * C], mybir.dt.float32, kind="ExternalInput").ap()
    offs = nc.dram_tensor("offs", [P, NBLK], mybir.dt.int32, kind="ExternalInput").ap()
    out = nc.dram_tensor("out", [BS * C], mybir.dt.float32, kind="ExternalOutput").ap()

    lg2 = lg.rearrange("(r c) -> r c", c=C)
    out2 = out.rearrange("(r c) -> r c", c=C)
    lgN1 = lg.rearrange("(n one) -> n one", one=1)
    outN1 = out.rearrange("(n one) -> n one", one=1)

    fp32 = mybir.dt.float32
    X = mybir.AxisListType.X

    with tile.TileContext(nc) as tc:
        with (
            tc.tile_pool(name="data", bufs=DATA_BUFS) as data,
            tc.tile_pool(name="stats", bufs=2) as stats,
            tc.tile_pool(name="singles", bufs=1) as singles,
        ):
            offs_t = singles.tile([P, NBLK], mybir.dt.int32)
            nc.sync.dma_start(out=offs_t[:], in_=offs[:])
            # Gather t = logits[flat_offset] for every block up front; only
            # needs the offsets, so it runs while the first loads stream in.
            t_all = singles.tile([P, NBLK], fp32)
            for b in range(NBLK):
                nc.gpsimd.indirect_dma_start(
                    out=t_all[:, b : b + 1],
                    out_offset=None,
                    in_=lgN1[:],
                    in_offset=bass.IndirectOffsetOnAxis(
                        ap=offs_t[:, b : b + 1], axis=0
                    ),
                )

            for b in range(NBLK):
                rows = slice(b * P, (b + 1) * P)
                sparts = stats.tile([P, NCH], fp32)
                chunks = []
                for k in range(NCH):
                    ck = data.tile([P, F], fp32, tag="data")
                    nc.sync.dma_start(
                        out=ck[:], in_=lg2[rows, k * F : (k + 1) * F]
                    )
                    nc.vector.reduce_sum(out=sparts[:, k : k + 1], in_=ck[:], axis=X)
                    chunks.append(ck)

                S = stats.tile([P, 1], fp32)
                nc.vector.reduce_sum(out=S[:], in_=sparts[:], axis=X)

                # s = ALPHA / (1 + S - 2 t)  ==  1 / ((1+S)/ALPHA - (2/ALPHA) t)
                e1 = stats.tile([P, 1], fp32)
                nc.vector.tensor_scalar(
                    out=e1[:], in0=S[:], scalar1=1.0 / ALPHA, scalar2=1.0 / ALPHA,
                    op0=mybir.AluOpType.mult, op1=mybir.AluOpType.add,
                )
                d1 = stats.tile([P, 1], fp32)
                nc.vector.tensor_scalar(
                    out=d1[:], in0=t_all[:, b : b + 1], scalar1=-2.0 / ALPHA,
                    scalar2=e1[:],
                    op0=mybir.AluOpType.mult, op1=mybir.AluOpType.add,
                )
                s_t = stats.tile([P, 1], fp32)
                nc.vector.reciprocal(out=s_t[:], in_=d1[:])

                # val = s*t + (1 - s*S)   (the corrected out[i, label])
                sS = stats.tile([P, 1], fp32)
                nc.vector.tensor_mul(out=sS[:], in0=s_t[:], in1=S[:])
                corr = stats.tile([P, 1], fp32)
                nc.vector.tensor_scalar(
                    out=corr[:], in0=sS[:], scalar1=-1.0, scalar2=1.0,
                    op0=mybir.AluOpType.mult, op1=mybir.AluOpType.add,
                )
                val = stats.tile([P, 1], fp32)
                nc.vector.tensor_scalar(
                    out=val[:], in0=t_all[:, b : b + 1], scalar1=s_t[:],
                    scalar2=corr[:],
                    op0=mybir.AluOpType.mult, op1=mybir.AluOpType.add,
                )

                store_insts = []
                for k, ck in enumerate(chunks):
                    nc.scalar.mul(out=ck[:], in_=ck[:], mul=s_t[:])
                    si = nc.scalar.dma_start(
                        out=out2[rows, k * F : (k + 1) * F], in_=ck[:]
                    )
                    store_insts.append(si)

                sc = nc.gpsimd.indirect_dma_start(
                    out=outN1[:],
                    out_offset=bass.IndirectOffsetOnAxis(
                        ap=offs_t[:, b : b + 1], axis=0
                    ),
                    in_=val[:],
                    in_offset=None,
                )
                # The scatter overwrites out[i, label] after the bulk store of
                # the same rows; Tile doesn't order DRAM WAW, so force it.
                for si in store_insts:
                    add_dep_helper(
                        _raw(sc), _raw(si), reason="label fixup after bulk store"
                    )

    nc.compile()
    return nc


def _get_nc():
    if "nc" not in _CACHE:
        _CACHE["nc"] = _build()
    return _CACHE["nc"]


def _shard(teacher_logits, true_labels):
    lg = np.asarray(teacher_logits, dtype=np.float32)
    lab = np.asarray(true_labels).astype(np.int64)
    assert lg.shape == (B, C) and lab.shape == (B,)
    local_rows = np.arange(BS, dtype=np.int64)
    in_maps = []
    for c in range(N_CORES):
        shard = np.ascontiguousarray(lg[c * BS : (c + 1) * BS]).reshape(-1)
        flat = local_rows * C + lab[c * BS : (c + 1) * BS]
        offs_mat = np.ascontiguousarray(
            flat.astype(np.int32).reshape(NBLK, P).T
        )
        in_maps.append({"logits": shard, "offs": offs_mat})
    return in_maps


def _run(teacher_logits, true_labels, **kwargs):
    nc = _get_nc()
    in_maps = _shard(teacher_logits, true_labels)
    res = run_bass_kernel_spmd(nc, in_maps, core_ids=list(range(N_CORES)), **kwargs)
    out = np.concatenate(
        [res.results[c]["out"].reshape(BS, C) for c in range(N_CORES)], axis=0
    )
    return out, res


def kernel(teacher_logits, true_labels):
    return _run(teacher_logits, true_labels)[0]


if __name__ == "__main__":
    rng = np.random.default_rng(0)
    lg = rng.random((B, C), dtype=np.float32)
    lab = rng.integers(0, C, size=(B,), dtype=np.int64)
    got = kernel(lg, lab)
    S = lg.sum(axis=1)
    t = lg[np.arange(B), lab]
    s = ALPHA / (1.0 + S - 2.0 * t)
    want = s[:, None] * lg
    want[np.arange(B), lab] += 1.0 - s * S
    err = np.abs(got - want).max() / np.abs(want).max()
    print("self-check rel err:", err)
